# revision 32
# baseline (speedup 1.0000x reference)
"""Trainium2 Bass kernel for multi-head attention with RoPE.

Problem: b=8, n=1024, d_model=768, heads=12, dim_head=64.
Strategy: data parallel over batch — each of the 8 NeuronCores handles one
batch element end-to-end (QKV proj + RoPE + attention + out proj). No
collectives needed.

Per-core math (all in transposed [feature, token] layout so every matmul
contraction sits on the partition axis):
  xT   [768,1024]  = x^T             (bf16, pre-transposed on host)
  qT/kT [128,1024] per head pair (2 heads x 64 rows), RoPE'd in bf16.
    rotate_half done via SBUF->SBUF DMA partition swap (qs[d] = qf[d^32]),
    so RoPE is 3 full-tile DVE ops (qa=qf*cos, qb=qs*sins, add) instead of
    4 partition-shifted quadrant muls (DVE time is free-size-bound, so the
    old [32,1024] muls each cost as much as a full tile).
  V    [1024,12*128] = x Wv, 128 cols/head: 64 v | ones col | zeros
  S^T per head pair via two CONCURRENT K=64 row-tiled matmuls (PE array
    row groups 0-63 / 64-127 run in parallel; tile_position auto-derived
    from the operands' base partitions).
  pT  = exp(sT / 8)                (no max-subtraction; |S/8| <~ 6)
  oT[128,1024] += PV accum over j tiles; row 64 = softmax denominators
                  (ones column), rows 65+ garbage (never read)
  aT = oT[0:64] * bcast(1/oT[64])  (recip_approx direct from PSUM rows +
                                    DRAM-broadcast DMA)
  out [1024,768] = aT^T Wout + b
"""

import os
import numpy as np
import ml_dtypes

N = 1024
D = 768
H = 12
DH = 64
E3 = 2304
KT = 6          # number of 128-row tiles of the model dim (768/128)
NT = 8          # number of 128-token tiles (1024/128)
P = 128
N_CORES = 8
VW = 65         # per-head V width incl. ones column

_CACHE = {}


def _build():
    import concourse.bass as bass
    import concourse.mybir as mybir
    import concourse.tile as tile
    from concourse import bacc

    F32 = mybir.dt.float32
    BF16 = mybir.dt.bfloat16
    Exp = mybir.ActivationFunctionType.Exp

    nc = bacc.Bacc("TRN2", target_bir_lowering=False, debug=False,
                   num_devices=N_CORES)

    x = nc.dram_tensor("x", [D, N], BF16, kind="ExternalInput")
    wqkv = nc.dram_tensor("wqkv", [P, KT * 1536], BF16, kind="ExternalInput")
    wv_d = nc.dram_tensor("wv_d", [P, KT * D], BF16, kind="ExternalInput")
    wout = nc.dram_tensor("wout", [P, KT * D], BF16, kind="ExternalInput")
    cos2 = nc.dram_tensor("cos2", [P, N], BF16, kind="ExternalInput")
    sins2 = nc.dram_tensor("sins2", [P, N], BF16, kind="ExternalInput")
    biasb = nc.dram_tensor("biasb", [P, D], F32, kind="ExternalInput")
    out = nc.dram_tensor("out", [N, D], F32, kind="ExternalOutput")
    DBG = os.environ.get("KERNEL_DBG", "0") == "1"
    if DBG:
        dbg_q = nc.dram_tensor("dbg_q", [P, N], BF16, kind="ExternalOutput")
        dbg_k = nc.dram_tensor("dbg_k", [P, N], BF16, kind="ExternalOutput")
        dbg_s = nc.dram_tensor("dbg_s", [P, N], F32, kind="ExternalOutput")
        dbg_s2 = nc.dram_tensor("dbg_s2", [P, N], F32, kind="ExternalOutput")
        dbg_rb = nc.dram_tensor("dbg_rb", [P, N], F32, kind="ExternalOutput")
        dbg_a0 = nc.dram_tensor("dbg_a0", [P, N], BF16, kind="ExternalOutput")

    with tile.TileContext(nc, pool_alloc_mode="queue") as tc:
        import contextlib
        with contextlib.ExitStack() as ctx:
            persist = ctx.enter_context(tc.tile_pool(name="persist", bufs=1))
            scr = ctx.enter_context(tc.tile_pool(name="scr", bufs=6))
            ptp = ctx.enter_context(tc.tile_pool(name="ptp", bufs=4))
            smallp = ctx.enter_context(tc.tile_pool(name="smallp", bufs=2))
            otp = ctx.enter_context(tc.tile_pool(name="otp", bufs=4))
            outp = ctx.enter_context(tc.tile_pool(name="outp", bufs=2))
            dramp = ctx.enter_context(
                tc.tile_pool(name="dram", bufs=2, space="DRAM"))

            # ---- startup loads: interleave x tiles with per-k wv chunks
            # so the first V matmul's deps (xT[0] + wv k=0) land in ~3us
            # instead of waiting for the whole 1.2MB wv DMA.
            xT = [persist.tile([P, N], BF16, tag=f"xT{t_i}",
                               name=f"xT_sb{t_i}") for t_i in range(KT)]
            wv_sb = persist.tile([P, KT * D], BF16, tag="wv", name="wv_sb")
            for t_i in range(KT):
                nc.sync.dma_start(xT[t_i][:], x[t_i * P:(t_i + 1) * P, :])
                nc.sync.dma_start(wv_sb[:, t_i * D:(t_i + 1) * D],
                                  wv_d[:, t_i * D:(t_i + 1) * D])
            wqk_sb = persist.tile([P, KT * 1536], BF16, tag="wqk",
                                  name="wqk_sb")
            for t_i in range(KT):
                nc.sync.dma_start(wqk_sb[:, t_i * 1536:(t_i + 1) * 1536],
                                  wqkv[:, t_i * 1536:(t_i + 1) * 1536])
            cos_sb = persist.tile([P, N], BF16, tag="cos", name="cos_sb")
            nc.sync.dma_start(cos_sb[:], cos2[:, :])
            sin_sb = persist.tile([P, N], BF16, tag="sin", name="sin_sb")
            nc.sync.dma_start(sin_sb[:], sins2[:, :])
            wo_sb = persist.tile([P, KT * D], BF16, tag="wo", name="wo_sb")
            nc.sync.dma_start(wo_sb[:], wout[:, :])
            bias_sb = persist.tile([P, D], F32, tag="bias", name="bias_sb")
            nc.sync.dma_start(bias_sb[:], biasb[:, :])

            # =======================================================
            # Phase B/C: projections (shared PSUM pool, closed after)
            # =======================================================
            # qkT[m]: m<6 -> q rows for heads 2m,2m+1; m>=6 -> roped k rows
            # for heads 2(m-6), 2(m-6)+1. Both in identical pair layout
            # (head u's 64 rows at partition base 64u) — the S matmul
            # row-tiles on the 64-row halves directly.
            qkT = [persist.tile([P, N], BF16, tag=f"qkT{m}", name=f"qkT_sb{m}")
                   for m in range(12)]
            # V tiles padded to 128 cols/head: 64 v-dims | ones | zeros,
            # so PV matmuls write a full M=128 (ones col -> sums row 64).
            vt = [persist.tile([P, H * P], BF16, tag=f"vt{n}", name=f"vt_sb{n}")
                  for n in range(NT)]
            aT = [persist.tile([P, N], BF16, tag=f"aT{e}", name=f"aT_sb{e}")
                  for e in range(KT)]

            # Two dedicated PSUM pools (2 slots x 2 banks each = all 8
            # banks): psS cycles projection / S^T / final tiles, psO holds
            # the two PV accumulators.
            with (tc.tile_pool(name="psS", bufs=2, space="PSUM") as psum,
                  tc.tile_pool(name="psO", bufs=2, space="PSUM") as psumO):
                # ---- V projection into per-head 65-wide layout ----
                for ni in range(NT):
                    vpool, vtag = ((psum, "ps") if ni % 2 == 0
                                   else (psumO, "ops"))
                    ps = vpool.tile([P, N], F32, tag=vtag, name="ps_v")
                    # K-contiguous (k innermost) — anything else makes
                    # the HAM clock-gate oscillate every k-tile (+25% mm)
                    for (c0, cw) in ((0, 512), (512, 256)):
                        for k in range(KT):
                            nc.tensor.matmul(
                                ps[:, c0:c0 + cw],
                                lhsT=xT[k][:, ni * P:(ni + 1) * P],
                                rhs=wv_sb[:, k * D + c0:k * D + c0 + cw],
                                start=(k == 0), stop=(k == KT - 1))
                    # scatter copy into head-strided slots (+ones col)
                    dst8 = vt[ni][:, 0:8 * P].rearrange(
                        "p (h j) -> p h j", j=P)[:, :, 0:DH]
                    src8 = ps[:, 0:512].rearrange("p (h j) -> p h j", j=DH)
                    nc.scalar.copy(dst8, src8)
                    dst4 = vt[ni][:, 8 * P:12 * P].rearrange(
                        "p (h j) -> p h j", j=P)[:, :, 0:DH]
                    src4 = ps[:, 512:768].rearrange("p (h j) -> p h j", j=DH)
                    nc.scalar.copy(dst4, src4)
                    vre = vt[ni].rearrange("p (h j) -> p h j", j=P)
                    nc.gpsimd.memset(vre[:, :, DH:DH + 1], 1.0)
                    nc.gpsimd.memset(vre[:, :, DH + 1:P], 0.0)

                # ---- q/k projection + RoPE helper. Upfront tiles use the
                # alternating pools with the cast on ACT (idle then);
                # boundary tiles (emitted between attention head pairs,
                # inside the psO drain window) use psO with the cast on
                # GpSimd so neither ACT's exp stream nor DVE stalls.
                def emit_qk_tile(m, qpool, qtag, cast_engine):
                    ps = qpool.tile([P, N], F32, tag=qtag, name="ps_qk")
                    for ih in range(2):
                        for k in range(KT):
                            nc.tensor.matmul(
                                ps[:, ih * 512:(ih + 1) * 512],
                                lhsT=wqk_sb[:, k * 1536 + m * P:
                                            k * 1536 + (m + 1) * P],
                                rhs=xT[k][:, ih * 512:(ih + 1) * 512],
                                start=(k == 0), stop=(k == KT - 1))
                    # RoPE in bf16: cast, DMA-swap 32-blocks (rotate_half
                    # on the partition axis, bounced through DRAM), then
                    # 3 full-tile DVE ops.
                    qf = scr.tile([P, N], BF16, tag="qf", name="qf_t")
                    if cast_engine == "act":
                        nc.scalar.copy(qf[:], ps[:])
                    else:
                        nc.vector.tensor_copy(qf[:], ps[:])
                    qd = dramp.tile([P, N], BF16, tag="qd", name="qd_t")
                    nc.sync.dma_start(qd[:], qf[:])
                    qs = scr.tile([P, N], BF16, tag="qs", name="qs_t")
                    for blk in range(4):
                        ob = blk * 32
                        ib = (blk ^ 1) * 32  # 0<->32, 64<->96
                        nc.gpsimd.dma_start(qs[ob:ob + 32, :],
                                            qd[ib:ib + 32, :])
                    qa = scr.tile([P, N], BF16, tag="qa", name="qa_t")
                    nc.vector.tensor_mul(qa[:], qf[:], cos_sb[:])
                    qb = scr.tile([P, N], BF16, tag="qb", name="qb_t")
                    nc.vector.tensor_mul(qb[:], qs[:], sin_sb[:])
                    nc.vector.tensor_add(qkT[m][:], qa[:], qb[:])

                # all 12 m-tiles upfront: inserting projection matmuls
                # into the attention stream stalls ACT (PE queue is strict
                # FIFO) and the idle gaps drop the PE clock to 4/8, so
                # keeping one dense PE phase is fastest.
                for mi, m in enumerate(
                        [t for hp in range(6) for t in (hp, 6 + hp)]):
                    qpool, qtag = ((psum, "ps") if mi % 2 == 0
                                   else (psumO, "ops"))
                    emit_qk_tile(m, qpool, qtag, "act")
                boundary_qk = {}

                if DBG:
                    nc.sync.dma_start(dbg_q[:, :], qkT[0][:])
                    nc.sync.dma_start(dbg_k[:, :], qkT[6][:])
                    dbg_s_sb = persist.tile([P, N], F32, tag="dbgs",
                                            name="dbgs_sb")
                ss_sb = persist.tile([33, N], F32, tag="ss", name="ss_t")
                nc.gpsimd.memset(ss_sb[0:32, :], 1.0)
                # ones row + bf16 bias row: lets even out-proj tiles fold
                # the bias add into the matmul group (K=1 ones-row mm) and
                # drain on ACT, halving the serial DVE add chain in the tail
                ones1 = persist.tile([1, P], BF16, tag="ones1", name="ones1_t")
                nc.gpsimd.memset(ones1[:], 1.0)
                bias_bf = persist.tile([1, D], BF16, tag="biasbf",
                                       name="biasbf_t")
                nc.vector.tensor_copy(bias_bf[:], bias_sb[0:1, :])

                # ---- attention, two heads (one row-group pair) at a
                # time; software-pipelined one step: emit S/exp of step
                # g+1 before PV of step g so the PE never stalls on the
                # last exp of a head pair (incl. across pair boundaries).
                o_ps_all = [[None, None] for _ in range(6)]

                def emit_s_exp(hp, j):
                    qt = qkT[hp]
                    kt = qkT[6 + hp]
                    s_ps = [psum.tile([P, N], F32, tag="ps",
                                      name=f"s_ps{u}") for u in range(2)]
                    for ih in range(2):
                        for u in range(2):  # u: head parity (row group)
                            nc.tensor.matmul(
                                s_ps[u][:, ih * 512:(ih + 1) * 512],
                                lhsT=kt[u * DH:(u + 1) * DH,
                                        j * P:(j + 1) * P],
                                rhs=qt[u * DH:(u + 1) * DH,
                                       ih * 512:(ih + 1) * 512],
                                start=True, stop=True)
                    if DBG and hp == 0 and j == 0:
                        nc.vector.tensor_copy(dbg_s_sb[:], s_ps[0][:])
                        nc.sync.dma_start(dbg_s[:, :], dbg_s_sb[:])
                    if DBG and hp == 0 and j == 1:
                        nc.vector.tensor_copy(dbg_s_sb[:], s_ps[1][:])
                        nc.sync.dma_start(dbg_s2[:, :], dbg_s_sb[:])
                    pT = [None, None]
                    for u in range(2):
                        pT[u] = ptp.tile([P, N], BF16, tag="pT",
                                         name=f"pT_t{u}")
                        nc.scalar.activation(pT[u][:], s_ps[u][:], Exp,
                                             scale=0.125)
                    return pT

                def emit_pv(hp, j, pT):
                    o_ps = o_ps_all[hp]
                    if j == 0:
                        o_ps[0] = psumO.tile([P, N], F32, tag="ops",
                                             name="o_ps0")
                        o_ps[1] = psumO.tile([P, N], F32, tag="ops",
                                             name="o_ps1")
                    for u in range(2):
                        h = 2 * hp + u
                        for ih in range(2):
                            nc.tensor.matmul(
                                o_ps[u][:, ih * 512:(ih + 1) * 512],
                                lhsT=vt[j][:, h * P:(h + 1) * P],
                                rhs=pT[u][:, ih * 512:(ih + 1) * 512],
                                start=(j == 0), stop=(j == NT - 1))
                    if j == NT - 1:
                        return emit_drain(hp, o_ps)

                def emit_drain(hp, o_ps):
                    # copy attn rows + denominator rows out of PSUM fast
                    # (frees psO for the boundary qk-proj tile + the next
                    # pair's PV). The recip custom-DVE op needs SBUF input
                    # — staging through ss_sb is required, not an
                    # optimization.
                    oTc = otp.tile([P, N], BF16, tag="oT", name="oT_t")
                    if hp == 5:
                        # last pair: the reciprocal chain gates phase E —
                        # drain the denominator rows and start the recip +
                        # broadcast DMA before the oTc copies (which then
                        # overlap the DMA round trip)
                        nc.vector.tensor_copy(ss_sb[0:1, :],
                                              o_ps[0][DH:DH + 1, :])
                        nc.vector.tensor_copy(ss_sb[32:33, :],
                                              o_ps[1][DH:DH + 1, :])
                        emit_recip_bcast()
                        nc.scalar.copy(oTc[0:DH, :], o_ps[0][0:DH, :])
                        nc.vector.tensor_copy(oTc[DH:P, :],
                                              o_ps[1][0:DH, :])
                    else:
                        nc.vector.tensor_copy(oTc[0:DH, :],
                                              o_ps[0][0:DH, :])
                        nc.vector.tensor_copy(ss_sb[0:1, :],
                                              o_ps[0][DH:DH + 1, :])
                        nc.vector.tensor_copy(oTc[DH:P, :],
                                              o_ps[1][0:DH, :])
                        nc.vector.tensor_copy(ss_sb[32:33, :],
                                              o_ps[1][DH:DH + 1, :])
                    return oTc

                def emit_recip_bcast():
                    # 1/sums + DRAM broadcast; result tile left in _rb
                    r_sb = smallp.tile([33, N], F32, tag="r", name="r_t")
                    nc.vector.reciprocal_approx_fast(r_sb[:], ss_sb[:])
                    r_dr = dramp.tile([2, N], F32, tag="rdr", name="rdr_t")
                    nc.sync.dma_start(r_dr[0:1, :], r_sb[0:1, :])
                    nc.sync.dma_start(r_dr[1:2, :], r_sb[32:33, :])
                    rb_sb = smallp.tile([P, N], F32, tag="rb", name="rb_t")
                    nc.sync.dma_start(rb_sb[0:DH, :],
                                      r_dr[0:1, :].broadcast_to([DH, N]))
                    nc.sync.dma_start(rb_sb[DH:P, :],
                                      r_dr[1:2, :].broadcast_to([DH, N]))
                    _rb[0] = rb_sb

                _rb = [None]

                def emit_norm_finish(hp, oTc):
                    # off the critical path for hp<5 (aT only feeds phase E)
                    if hp != 5:
                        emit_recip_bcast()
                    nc.vector.tensor_mul(aT[hp][:], oTc[:], _rb[0][:])
                    if DBG and hp == 0:
                        nc.sync.dma_start(dbg_rb[:, :], rb_sb[:])
                        nc.sync.dma_start(dbg_a0[:, :], aT[0][:])

                steps = [(hp, j) for hp in range(6) for j in range(NT)]
                prev = None
                for st in steps:
                    pT = emit_s_exp(*st)
                    if prev is not None:
                        oTc = emit_pv(*prev)
                        if prev[1] == NT - 1:
                            # pair boundary: o_ps(hp) just drained — slip
                            # the next-next pair's q/k projection into the
                            # freed psO slots (their DVE casts emitted
                            # before the normalize finish so the slots
                            # release before PV(hp+1, 0) needs them).
                            for m in boundary_qk.get(prev[0], ()):
                                emit_qk_tile(m, psumO, "ops", "dve")
                            emit_norm_finish(prev[0], oTc)
                    prev = (st[0], st[1], pT)
                oTc = emit_pv(*prev)

                # ---- output projection + bias. The first 4 it-tiles
                # accumulate their e<=4 contributions while the last
                # pair's normalize (recip + DMA broadcast round trip) is
                # in flight — keeps the PE warm through the phase gap and
                # starts phase E ~5us earlier.
                def emit_out_mms(f_ps, it, es, start0, stop5):
                    for (c0, cw) in ((0, 512), (512, 256)):
                        for e in es:
                            nc.tensor.matmul(
                                f_ps[:, c0:c0 + cw],
                                lhsT=aT[e][:, it * P:(it + 1) * P],
                                rhs=wo_sb[:, e * D + c0:e * D + c0 + cw],
                                start=(e == 0 and start0),
                                stop=(e == KT - 1 and stop5))

                def emit_out_finish(f_ps, it):
                    # even tiles: bias via K=1 ones-row mm + drain on ACT;
                    # odd tiles: bias in the DVE add. The two chains run
                    # on different engines so tile completions overlap.
                    o_sb = outp.tile([P, D], F32, tag="osb", name="osb_t")
                    if it % 2 == 0:
                        for (c0, cw) in ((0, 512), (512, 256)):
                            nc.tensor.matmul(
                                f_ps[:, c0:c0 + cw],
                                lhsT=ones1[:, :],
                                rhs=bias_bf[:, c0:c0 + cw],
                                start=False, stop=True)
                        nc.scalar.copy(o_sb[:], f_ps[:, 0:D])
                    else:
                        nc.vector.tensor_add(o_sb[:], f_ps[:, 0:D],
                                             bias_sb[:])
                    nc.sync.dma_start(out[it * P:(it + 1) * P, :], o_sb[:])

                f_head = {}
                for it in range(4):
                    f_pool = psumO if it % 2 == 0 else psum
                    f_ps = f_pool.tile([P, N], F32,
                                       tag="ops" if it % 2 == 0 else "ps",
                                       name="f_ps")
                    f_head[it] = f_ps
                    emit_out_mms(f_ps, it, range(KT - 1), True, False)

                emit_norm_finish(5, oTc)

                for it in range(NT):
                    stop5 = it % 2 == 1  # even tiles stop on the bias mm
                    if it < 4:
                        f_ps = f_head[it]
                        emit_out_mms(f_ps, it, [KT - 1], False, stop5)
                    else:
                        f_pool = psumO if it % 2 == 0 else psum
                        f_ps = f_pool.tile([P, N], F32,
                                           tag="ops" if it % 2 == 0 else "ps",
                                           name="f_ps")
                        emit_out_mms(f_ps, it, range(KT), True, stop5)
                    emit_out_finish(f_ps, it)

    nc.compile()
    return nc


def _host_tables():
    inv_freq = 1.0 / (10000.0 ** (np.arange(0, DH, 2, dtype=np.float32) / DH))
    t = np.arange(N, dtype=np.float32)
    freqs = np.einsum("i,j->ij", t, inv_freq)          # [N, 32]
    emb = np.concatenate([freqs, freqs], axis=-1)      # [N, 64]
    cosT = np.cos(emb).T.astype(np.float32)            # [64, N]
    sinT = np.sin(emb).T.astype(np.float32)            # [64, N]
    # b-term: out rows 0:32 use -sin (pair d+32), rows 32:64 use +sin.
    # No 32-block pre-swap: the device builds qs[d] = qf[d^32] via DMA,
    # so the sin table is indexed by the OUTPUT row directly.
    sins = np.concatenate([-sinT[0:32], sinT[32:64]], axis=0)  # [64, N]
    cos2 = np.concatenate([cosT, cosT], axis=0)        # [128, N]
    sins2 = np.concatenate([sins, sins], axis=0)       # [128, N]
    return np.ascontiguousarray(cos2), np.ascontiguousarray(sins2)


def kernel(x, w_qkv, w_out, b_out):
    from concourse.bass_utils import run_bass_kernel_spmd

    if "nc" not in _CACHE:
        _CACHE["nc"] = _build()
    nc = _CACHE["nc"]

    bf = ml_dtypes.bfloat16
    cos2, sins2 = _host_tables()
    cos2 = np.ascontiguousarray(cos2.astype(bf))
    sins2 = np.ascontiguousarray(sins2.astype(bf))
    biasb = np.ascontiguousarray(
        np.broadcast_to(np.asarray(b_out, np.float32)[None, :], (P, D)))
    def _sbufize(w):   # [(k p), e] -> [p, (k e)] exact SBUF layout
        w = np.asarray(w, np.float32).astype(bf)
        k, e = w.shape[0] // P, w.shape[1]
        return np.ascontiguousarray(
            w.reshape(k, P, e).transpose(1, 0, 2).reshape(P, k * e))
    wqkv_b = _sbufize(np.asarray(w_qkv, np.float32)[:, 0:1536])
    wv_b = _sbufize(np.asarray(w_qkv, np.float32)[:, 1536:E3])
    wout_b = _sbufize(w_out)

    in_maps = []
    for i in range(N_CORES):
        xi = np.ascontiguousarray(
            np.asarray(x[i], np.float32).astype(bf).T)
        in_maps.append({
            "x": xi, "wqkv": wqkv_b, "wv_d": wv_b, "wout": wout_b,
            "cos2": cos2, "sins2": sins2, "biasb": biasb,
        })

    res = run_bass_kernel_spmd(
        nc, in_maps, list(range(N_CORES)),
        trace=bool(int(os.environ.get("KERNEL_TRACE", "0"))))
    _CACHE["last_result"] = res
    return np.stack([res.results[i]["out"] for i in range(N_CORES)], axis=0)


# revision 34
# speedup vs baseline: 1.2509x; 1.2509x over previous
"""Trainium2 Bass kernel for multi-head attention with RoPE.

Problem: b=8, n=1024, d_model=768, heads=12, dim_head=64.
Strategy: data parallel over batch — each of the 8 NeuronCores handles one
batch element end-to-end (QKV proj + RoPE + attention + out proj). No
collectives needed.

Per-core math (all in transposed [feature, token] layout so every matmul
contraction sits on the partition axis):
  xT   [768,1024]  = x^T             (bf16, pre-transposed on host)
  qT/kT [128,1024] per head pair (2 heads x 64 rows), RoPE'd in bf16.
    rotate_half done via SBUF->SBUF DMA partition swap (qs[d] = qf[d^32]),
    so RoPE is 3 full-tile DVE ops (qa=qf*cos, qb=qs*sins, add) instead of
    4 partition-shifted quadrant muls (DVE time is free-size-bound, so the
    old [32,1024] muls each cost as much as a full tile).
  V    [1024,12*128] = x Wv, 128 cols/head: 64 v | ones col | zeros
  S^T per head pair via two CONCURRENT K=64 row-tiled matmuls (PE array
    row groups 0-63 / 64-127 run in parallel; tile_position auto-derived
    from the operands' base partitions).
  pT  = exp(sT / 8)                (no max-subtraction; |S/8| <~ 6)
  oT[128,1024] += PV accum over j tiles; row 64 = softmax denominators
                  (ones column), rows 65+ garbage (never read)
  aT = oT[0:64] * bcast(1/oT[64])  (recip_approx direct from PSUM rows +
                                    DRAM-broadcast DMA)
  out [1024,768] = aT^T Wout + b
"""

import os
import numpy as np
import ml_dtypes

N = 1024
D = 768
H = 12
DH = 64
E3 = 2304
KT = 6          # number of 128-row tiles of the model dim (768/128)
NT = 8          # number of 128-token tiles (1024/128)
P = 128
N_CORES = 8
VW = 65         # per-head V width incl. ones column

_CACHE = {}


def _build():
    import concourse.bass as bass
    import concourse.mybir as mybir
    import concourse.tile as tile
    from concourse import bacc

    F32 = mybir.dt.float32
    BF16 = mybir.dt.bfloat16
    Exp = mybir.ActivationFunctionType.Exp

    nc = bacc.Bacc("TRN2", target_bir_lowering=False, debug=False,
                   num_devices=N_CORES)

    x = nc.dram_tensor("x", [D, N], BF16, kind="ExternalInput")
    wqkv = nc.dram_tensor("wqkv", [P, KT * 1536], BF16, kind="ExternalInput")
    wv_d = nc.dram_tensor("wv_d", [P, KT * D], BF16, kind="ExternalInput")
    wout = nc.dram_tensor("wout", [P, KT * D], BF16, kind="ExternalInput")
    cos2 = nc.dram_tensor("cos2", [P, N], BF16, kind="ExternalInput")
    sins2 = nc.dram_tensor("sins2", [P, N], BF16, kind="ExternalInput")
    biasb = nc.dram_tensor("biasb", [P, D], F32, kind="ExternalInput")
    out = nc.dram_tensor("out", [N, D], F32, kind="ExternalOutput")
    DBG = os.environ.get("KERNEL_DBG", "0") == "1"
    if DBG:
        dbg_q = nc.dram_tensor("dbg_q", [P, N], BF16, kind="ExternalOutput")
        dbg_k = nc.dram_tensor("dbg_k", [P, N], BF16, kind="ExternalOutput")
        dbg_s = nc.dram_tensor("dbg_s", [P, N], F32, kind="ExternalOutput")
        dbg_s2 = nc.dram_tensor("dbg_s2", [P, N], F32, kind="ExternalOutput")
        dbg_rb = nc.dram_tensor("dbg_rb", [P, N], F32, kind="ExternalOutput")
        dbg_a0 = nc.dram_tensor("dbg_a0", [P, N], BF16, kind="ExternalOutput")

    with tile.TileContext(nc, pool_alloc_mode="queue") as tc:
        import contextlib
        with contextlib.ExitStack() as ctx:
            persist = ctx.enter_context(tc.tile_pool(name="persist", bufs=1))
            scr = ctx.enter_context(tc.tile_pool(name="scr", bufs=6))
            ptp = ctx.enter_context(tc.tile_pool(name="ptp", bufs=4))
            smallp = ctx.enter_context(tc.tile_pool(name="smallp", bufs=2))
            otp = ctx.enter_context(tc.tile_pool(name="otp", bufs=4))
            outp = ctx.enter_context(tc.tile_pool(name="outp", bufs=2))
            dramp = ctx.enter_context(
                tc.tile_pool(name="dram", bufs=2, space="DRAM"))

            # ---- startup loads: interleave x tiles with per-k wv chunks
            # so the first V matmul's deps (xT[0] + wv k=0) land in ~3us
            # instead of waiting for the whole 1.2MB wv DMA.
            xT = [persist.tile([P, N], BF16, tag=f"xT{t_i}",
                               name=f"xT_sb{t_i}") for t_i in range(KT)]
            wv_sb = persist.tile([P, KT * D], BF16, tag="wv", name="wv_sb")
            for t_i in range(KT):
                nc.sync.dma_start(xT[t_i][:], x[t_i * P:(t_i + 1) * P, :])
                nc.sync.dma_start(wv_sb[:, t_i * D:(t_i + 1) * D],
                                  wv_d[:, t_i * D:(t_i + 1) * D])
            wqk_sb = persist.tile([P, KT * 1536], BF16, tag="wqk",
                                  name="wqk_sb")
            for t_i in range(KT):
                nc.sync.dma_start(wqk_sb[:, t_i * 1536:(t_i + 1) * 1536],
                                  wqkv[:, t_i * 1536:(t_i + 1) * 1536])
            cos_sb = persist.tile([P, N], BF16, tag="cos", name="cos_sb")
            nc.sync.dma_start(cos_sb[:], cos2[:, :])
            sin_sb = persist.tile([P, N], BF16, tag="sin", name="sin_sb")
            nc.sync.dma_start(sin_sb[:], sins2[:, :])
            wo_sb = persist.tile([P, KT * D], BF16, tag="wo", name="wo_sb")
            nc.sync.dma_start(wo_sb[:], wout[:, :])
            bias_sb = persist.tile([P, D], F32, tag="bias", name="bias_sb")
            nc.sync.dma_start(bias_sb[:], biasb[:, :])

            # =======================================================
            # Phase B/C: projections (shared PSUM pool, closed after)
            # =======================================================
            # qkT[m]: m<6 -> q rows for heads 2m,2m+1; m>=6 -> roped k rows
            # for heads 2(m-6), 2(m-6)+1. Both in identical pair layout
            # (head u's 64 rows at partition base 64u) — the S matmul
            # row-tiles on the 64-row halves directly.
            qkT = [persist.tile([P, N], BF16, tag=f"qkT{m}", name=f"qkT_sb{m}")
                   for m in range(12)]
            # V tiles padded to 128 cols/head: 64 v-dims | ones | zeros,
            # so PV matmuls write a full M=128 (ones col -> sums row 64).
            vt = [persist.tile([P, H * P], BF16, tag=f"vt{n}", name=f"vt_sb{n}")
                  for n in range(NT)]
            aT = [persist.tile([P, N], BF16, tag=f"aT{e}", name=f"aT_sb{e}")
                  for e in range(KT)]

            # Two dedicated PSUM pools (2 slots x 2 banks each = all 8
            # banks): psS cycles projection / S^T / final tiles, psO holds
            # the two PV accumulators.
            with (tc.tile_pool(name="psS", bufs=2, space="PSUM") as psum,
                  tc.tile_pool(name="psO", bufs=2, space="PSUM") as psumO):
                # ---- V projection into per-head 65-wide layout ----
                for ni in range(NT):
                    vpool, vtag = ((psum, "ps") if ni % 2 == 0
                                   else (psumO, "ops"))
                    ps = vpool.tile([P, N], F32, tag=vtag, name="ps_v")
                    # K-contiguous (k innermost) — anything else makes
                    # the HAM clock-gate oscillate every k-tile (+25% mm)
                    for (c0, cw) in ((0, 512), (512, 256)):
                        for k in range(KT):
                            nc.tensor.matmul(
                                ps[:, c0:c0 + cw],
                                lhsT=xT[k][:, ni * P:(ni + 1) * P],
                                rhs=wv_sb[:, k * D + c0:k * D + c0 + cw],
                                start=(k == 0), stop=(k == KT - 1))
                    # scatter copy into head-strided slots (+ones col)
                    dst8 = vt[ni][:, 0:8 * P].rearrange(
                        "p (h j) -> p h j", j=P)[:, :, 0:DH]
                    src8 = ps[:, 0:512].rearrange("p (h j) -> p h j", j=DH)
                    nc.scalar.copy(dst8, src8)
                    dst4 = vt[ni][:, 8 * P:12 * P].rearrange(
                        "p (h j) -> p h j", j=P)[:, :, 0:DH]
                    src4 = ps[:, 512:768].rearrange("p (h j) -> p h j", j=DH)
                    nc.scalar.copy(dst4, src4)
                    vre = vt[ni].rearrange("p (h j) -> p h j", j=P)
                    nc.gpsimd.memset(vre[:, :, DH:DH + 1], 1.0)
                    nc.gpsimd.memset(vre[:, :, DH + 1:P], 0.0)

                # ---- q/k projection + RoPE helper. Upfront tiles use the
                # alternating pools with the cast on ACT (idle then);
                # boundary tiles (emitted between attention head pairs,
                # inside the psO drain window) use psO with the cast on
                # GpSimd so neither ACT's exp stream nor DVE stalls.
                def emit_qk_tile(m, qpool, qtag, cast_engine):
                    ps = qpool.tile([P, N], F32, tag=qtag, name="ps_qk")
                    for ih in range(2):
                        for k in range(KT):
                            nc.tensor.matmul(
                                ps[:, ih * 512:(ih + 1) * 512],
                                lhsT=wqk_sb[:, k * 1536 + m * P:
                                            k * 1536 + (m + 1) * P],
                                rhs=xT[k][:, ih * 512:(ih + 1) * 512],
                                start=(k == 0), stop=(k == KT - 1))
                    # RoPE in bf16: cast, DMA-swap 32-blocks (rotate_half
                    # on the partition axis, bounced through DRAM), then
                    # 3 full-tile DVE ops.
                    qf = scr.tile([P, N], BF16, tag="qf", name="qf_t")
                    if cast_engine == "act":
                        nc.scalar.copy(qf[:], ps[:])
                    else:
                        nc.vector.tensor_copy(qf[:], ps[:])
                    qd = dramp.tile([P, N], BF16, tag="qd", name="qd_t")
                    nc.sync.dma_start(qd[:], qf[:])
                    qs = scr.tile([P, N], BF16, tag="qs", name="qs_t")
                    for blk in range(4):
                        ob = blk * 32
                        ib = (blk ^ 1) * 32  # 0<->32, 64<->96
                        nc.gpsimd.dma_start(qs[ob:ob + 32, :],
                                            qd[ib:ib + 32, :])
                    qa = scr.tile([P, N], BF16, tag="qa", name="qa_t")
                    nc.vector.tensor_mul(qa[:], qf[:], cos_sb[:])
                    qb = scr.tile([P, N], BF16, tag="qb", name="qb_t")
                    nc.vector.tensor_mul(qb[:], qs[:], sin_sb[:])
                    nc.vector.tensor_add(qkT[m][:], qa[:], qb[:])

                # all 12 m-tiles upfront: inserting projection matmuls
                # into the attention stream stalls ACT (PE queue is strict
                # FIFO) and the idle gaps drop the PE clock to 4/8, so
                # keeping one dense PE phase is fastest.
                for mi, m in enumerate(
                        [t for hp in range(6) for t in (hp, 6 + hp)]):
                    qpool, qtag = ((psum, "ps") if mi % 2 == 0
                                   else (psumO, "ops"))
                    emit_qk_tile(m, qpool, qtag, "act")
                boundary_qk = {}

                if DBG:
                    nc.sync.dma_start(dbg_q[:, :], qkT[0][:])
                    nc.sync.dma_start(dbg_k[:, :], qkT[6][:])
                    dbg_s_sb = persist.tile([P, N], F32, tag="dbgs",
                                            name="dbgs_sb")
                ss_sb = persist.tile([33, N], F32, tag="ss", name="ss_t")
                nc.gpsimd.memset(ss_sb[0:32, :], 1.0)
                # ones row + bf16 bias row: lets even out-proj tiles fold
                # the bias add into the matmul group (K=1 ones-row mm) and
                # drain on ACT, halving the serial DVE add chain in the tail
                ones1 = persist.tile([1, P], BF16, tag="ones1", name="ones1_t")
                nc.gpsimd.memset(ones1[:], 1.0)
                bias_bf = persist.tile([1, D], BF16, tag="biasbf",
                                       name="biasbf_t")
                # (cast emitted in the tail — emitting it here would stall
                # the in-order DVE queue on the late bias DMA)

                # ---- attention, two heads (one row-group pair) at a
                # time; software-pipelined one step: emit S/exp of step
                # g+1 before PV of step g so the PE never stalls on the
                # last exp of a head pair (incl. across pair boundaries).
                o_ps_all = [[None, None] for _ in range(6)]

                def emit_s_exp(hp, j):
                    qt = qkT[hp]
                    kt = qkT[6 + hp]
                    s_ps = [psum.tile([P, N], F32, tag="ps",
                                      name=f"s_ps{u}") for u in range(2)]
                    for ih in range(2):
                        for u in range(2):  # u: head parity (row group)
                            nc.tensor.matmul(
                                s_ps[u][:, ih * 512:(ih + 1) * 512],
                                lhsT=kt[u * DH:(u + 1) * DH,
                                        j * P:(j + 1) * P],
                                rhs=qt[u * DH:(u + 1) * DH,
                                       ih * 512:(ih + 1) * 512],
                                start=True, stop=True)
                    if DBG and hp == 0 and j == 0:
                        nc.vector.tensor_copy(dbg_s_sb[:], s_ps[0][:])
                        nc.sync.dma_start(dbg_s[:, :], dbg_s_sb[:])
                    if DBG and hp == 0 and j == 1:
                        nc.vector.tensor_copy(dbg_s_sb[:], s_ps[1][:])
                        nc.sync.dma_start(dbg_s2[:, :], dbg_s_sb[:])
                    pT = [None, None]
                    for u in range(2):
                        pT[u] = ptp.tile([P, N], BF16, tag="pT",
                                         name=f"pT_t{u}")
                        nc.scalar.activation(pT[u][:], s_ps[u][:], Exp,
                                             scale=0.125)
                    return pT

                def emit_pv(hp, j, pT):
                    o_ps = o_ps_all[hp]
                    if j == 0:
                        o_ps[0] = psumO.tile([P, N], F32, tag="ops",
                                             name="o_ps0")
                        o_ps[1] = psumO.tile([P, N], F32, tag="ops",
                                             name="o_ps1")
                    for u in range(2):
                        h = 2 * hp + u
                        for ih in range(2):
                            nc.tensor.matmul(
                                o_ps[u][:, ih * 512:(ih + 1) * 512],
                                lhsT=vt[j][:, h * P:(h + 1) * P],
                                rhs=pT[u][:, ih * 512:(ih + 1) * 512],
                                start=(j == 0), stop=(j == NT - 1))
                    if j == NT - 1:
                        return emit_drain(hp, o_ps)

                def emit_drain(hp, o_ps):
                    # copy attn rows + denominator rows out of PSUM fast
                    # (frees psO for the boundary qk-proj tile + the next
                    # pair's PV). The recip custom-DVE op needs SBUF input
                    # — staging through ss_sb is required, not an
                    # optimization.
                    oTc = otp.tile([P, N], BF16, tag="oT", name="oT_t")
                    if hp == 5:
                        # last pair: the reciprocal chain gates phase E —
                        # drain the denominator rows and start the recip +
                        # broadcast DMA before the oTc copies (which then
                        # overlap the DMA round trip)
                        nc.vector.tensor_copy(ss_sb[0:1, :],
                                              o_ps[0][DH:DH + 1, :])
                        nc.vector.tensor_copy(ss_sb[32:33, :],
                                              o_ps[1][DH:DH + 1, :])
                        emit_recip_bcast()
                        nc.scalar.copy(oTc[0:DH, :], o_ps[0][0:DH, :])
                        nc.vector.tensor_copy(oTc[DH:P, :],
                                              o_ps[1][0:DH, :])
                    else:
                        nc.vector.tensor_copy(oTc[0:DH, :],
                                              o_ps[0][0:DH, :])
                        nc.vector.tensor_copy(ss_sb[0:1, :],
                                              o_ps[0][DH:DH + 1, :])
                        nc.vector.tensor_copy(oTc[DH:P, :],
                                              o_ps[1][0:DH, :])
                        nc.vector.tensor_copy(ss_sb[32:33, :],
                                              o_ps[1][DH:DH + 1, :])
                    return oTc

                def emit_recip_bcast():
                    # 1/sums + DRAM broadcast; result tile left in _rb
                    r_sb = smallp.tile([33, N], F32, tag="r", name="r_t")
                    nc.vector.reciprocal_approx_fast(r_sb[:], ss_sb[:])
                    r_dr = dramp.tile([2, N], F32, tag="rdr", name="rdr_t")
                    nc.sync.dma_start(r_dr[0:1, :], r_sb[0:1, :])
                    nc.sync.dma_start(r_dr[1:2, :], r_sb[32:33, :])
                    rb_sb = smallp.tile([P, N], F32, tag="rb", name="rb_t")
                    nc.sync.dma_start(rb_sb[0:DH, :],
                                      r_dr[0:1, :].broadcast_to([DH, N]))
                    nc.sync.dma_start(rb_sb[DH:P, :],
                                      r_dr[1:2, :].broadcast_to([DH, N]))
                    _rb[0] = rb_sb

                _rb = [None]

                def emit_norm_finish(hp, oTc):
                    # off the critical path for hp<5 (aT only feeds phase E)
                    if hp != 5:
                        emit_recip_bcast()
                    nc.vector.tensor_mul(aT[hp][:], oTc[:], _rb[0][:])
                    if DBG and hp == 0:
                        nc.sync.dma_start(dbg_rb[:, :], rb_sb[:])
                        nc.sync.dma_start(dbg_a0[:, :], aT[0][:])

                steps = [(hp, j) for hp in range(6) for j in range(NT)]
                prev = None
                for st in steps:
                    pT = emit_s_exp(*st)
                    if prev is not None:
                        oTc = emit_pv(*prev)
                        if prev[1] == NT - 1:
                            # pair boundary: o_ps(hp) just drained — slip
                            # the next-next pair's q/k projection into the
                            # freed psO slots (their DVE casts emitted
                            # before the normalize finish so the slots
                            # release before PV(hp+1, 0) needs them).
                            for m in boundary_qk.get(prev[0], ()):
                                emit_qk_tile(m, psumO, "ops", "dve")
                            emit_norm_finish(prev[0], oTc)
                    prev = (st[0], st[1], pT)
                oTc = emit_pv(*prev)
                nc.vector.tensor_copy(bias_bf[:], bias_sb[0:1, :])

                # ---- output projection + bias. The first 4 it-tiles
                # accumulate their e<=4 contributions while the last
                # pair's normalize (recip + DMA broadcast round trip) is
                # in flight — keeps the PE warm through the phase gap and
                # starts phase E ~5us earlier.
                def emit_out_mms(f_ps, it, es, start0, stop5):
                    for (c0, cw) in ((0, 512), (512, 256)):
                        for e in es:
                            nc.tensor.matmul(
                                f_ps[:, c0:c0 + cw],
                                lhsT=aT[e][:, it * P:(it + 1) * P],
                                rhs=wo_sb[:, e * D + c0:e * D + c0 + cw],
                                start=(e == 0 and start0),
                                stop=(e == KT - 1 and stop5))

                def emit_out_finish(f_ps, it):
                    # even tiles: bias via K=1 ones-row mm + drain on ACT;
                    # odd tiles: bias in the DVE add. The two chains run
                    # on different engines so tile completions overlap.
                    o_sb = outp.tile([P, D], F32, tag="osb", name="osb_t")
                    if it % 2 == 0:
                        for (c0, cw) in ((0, 512), (512, 256)):
                            nc.tensor.matmul(
                                f_ps[:, c0:c0 + cw],
                                lhsT=ones1[:, :],
                                rhs=bias_bf[:, c0:c0 + cw],
                                start=False, stop=True)
                        nc.scalar.copy(o_sb[:], f_ps[:, 0:D])
                    else:
                        nc.vector.tensor_add(o_sb[:], f_ps[:, 0:D],
                                             bias_sb[:])
                    nc.sync.dma_start(out[it * P:(it + 1) * P, :], o_sb[:])

                f_head = {}
                for it in range(4):
                    f_pool = psumO if it % 2 == 0 else psum
                    f_ps = f_pool.tile([P, N], F32,
                                       tag="ops" if it % 2 == 0 else "ps",
                                       name="f_ps")
                    f_head[it] = f_ps
                    emit_out_mms(f_ps, it, range(KT - 1), True, False)

                emit_norm_finish(5, oTc)

                for it in range(NT):
                    stop5 = it % 2 == 1  # even tiles stop on the bias mm
                    if it < 4:
                        f_ps = f_head[it]
                        emit_out_mms(f_ps, it, [KT - 1], False, stop5)
                    else:
                        f_pool = psumO if it % 2 == 0 else psum
                        f_ps = f_pool.tile([P, N], F32,
                                           tag="ops" if it % 2 == 0 else "ps",
                                           name="f_ps")
                        emit_out_mms(f_ps, it, range(KT), True, stop5)
                    emit_out_finish(f_ps, it)

    nc.compile()
    return nc


def _host_tables():
    inv_freq = 1.0 / (10000.0 ** (np.arange(0, DH, 2, dtype=np.float32) / DH))
    t = np.arange(N, dtype=np.float32)
    freqs = np.einsum("i,j->ij", t, inv_freq)          # [N, 32]
    emb = np.concatenate([freqs, freqs], axis=-1)      # [N, 64]
    cosT = np.cos(emb).T.astype(np.float32)            # [64, N]
    sinT = np.sin(emb).T.astype(np.float32)            # [64, N]
    # b-term: out rows 0:32 use -sin (pair d+32), rows 32:64 use +sin.
    # No 32-block pre-swap: the device builds qs[d] = qf[d^32] via DMA,
    # so the sin table is indexed by the OUTPUT row directly.
    sins = np.concatenate([-sinT[0:32], sinT[32:64]], axis=0)  # [64, N]
    cos2 = np.concatenate([cosT, cosT], axis=0)        # [128, N]
    sins2 = np.concatenate([sins, sins], axis=0)       # [128, N]
    return np.ascontiguousarray(cos2), np.ascontiguousarray(sins2)


def kernel(x, w_qkv, w_out, b_out):
    from concourse.bass_utils import run_bass_kernel_spmd

    if "nc" not in _CACHE:
        _CACHE["nc"] = _build()
    nc = _CACHE["nc"]

    bf = ml_dtypes.bfloat16
    cos2, sins2 = _host_tables()
    cos2 = np.ascontiguousarray(cos2.astype(bf))
    sins2 = np.ascontiguousarray(sins2.astype(bf))
    biasb = np.ascontiguousarray(
        np.broadcast_to(np.asarray(b_out, np.float32)[None, :], (P, D)))
    def _sbufize(w):   # [(k p), e] -> [p, (k e)] exact SBUF layout
        w = np.asarray(w, np.float32).astype(bf)
        k, e = w.shape[0] // P, w.shape[1]
        return np.ascontiguousarray(
            w.reshape(k, P, e).transpose(1, 0, 2).reshape(P, k * e))
    wqkv_b = _sbufize(np.asarray(w_qkv, np.float32)[:, 0:1536])
    wv_b = _sbufize(np.asarray(w_qkv, np.float32)[:, 1536:E3])
    wout_b = _sbufize(w_out)

    in_maps = []
    for i in range(N_CORES):
        xi = np.ascontiguousarray(
            np.asarray(x[i], np.float32).astype(bf).T)
        in_maps.append({
            "x": xi, "wqkv": wqkv_b, "wv_d": wv_b, "wout": wout_b,
            "cos2": cos2, "sins2": sins2, "biasb": biasb,
        })

    res = run_bass_kernel_spmd(
        nc, in_maps, list(range(N_CORES)),
        trace=bool(int(os.environ.get("KERNEL_TRACE", "0"))))
    _CACHE["last_result"] = res
    return np.stack([res.results[i]["out"] for i in range(N_CORES)], axis=0)
